# revision 1
# baseline (speedup 1.0000x reference)
"""APPNP (2-layer MLP + 2x K=10 personalized-pagerank propagation) on 8 TRN2 cores.

Strategy (constrained by what this runtime supports — see notes below):
- The two Linear+ReLU layers run on the 8 NeuronCores, row-sharded
  (12512 nodes per core), as a single compiled Bass/Tile program reused
  for both layers (K padded to 128, M padded to 64).
- The sparse propagation (segment-sum over 1.7M edges, x10 hops, x2
  layers) runs on host. On this runtime every batched-gather primitive
  is unusable: indirect_dma_start only honors one index per partition
  (and costs ~94us/call), and InstIndirectCopy / InstDMAGatherAnt /
  the Ant ext-isa GPSIMD family crash the device. Host-side
  sort+reduceat is exact and avoids per-hop device round-trips
  (each of which costs seconds of axon-tunnel upload).
- Normalization is folded: deg/dinv are computed once; self-loops are
  folded into the per-node update constants.

If anything in the device path fails, kernel() falls back to a pure
numpy implementation (identical math) so the result stays correct.
"""
import sys
import numpy as np

sys.path.insert(0, '/opt/trn_rl_repo')

N = 100000
E = 1600000
F_IN = 128
F_HID = 64
F_OUT = 40
K_HOPS = 10
ALPHA = 0.1

N_CORES = 8
ROWS_PAD = 100096          # N rounded up to 128*8*...: 100096 = 8 * 12512
ROWS_PER_CORE = ROWS_PAD // N_CORES   # 12512, = 97.75 -> pad to 98 blocks of 128
BLOCKS = ROWS_PER_CORE // 128         # 97.75 -> not integer; pad rows per core to 12544
ROWS_PER_CORE_PAD = 12544             # 98 * 128
KPAD = 128
MPAD = 64

_compiled = {}


def _build_gemm_relu():
    """One generic row-sharded GEMM+ReLU program: out = relu(x @ W + b).

    Inputs per core: xt [KPAD, ROWS_PER_CORE_PAD] (features-major, host
    pre-transposed shard), w [KPAD, MPAD], b [1, MPAD].
    Output: y [ROWS_PER_CORE_PAD, MPAD].
    """
    from concourse import bass, bacc, tile, mybir

    nc = bacc.Bacc("TRN2", target_bir_lowering=False, debug=False,
                   enable_asserts=True, num_devices=N_CORES)
    xt = nc.dram_tensor("xt", [KPAD, ROWS_PER_CORE_PAD], mybir.dt.float32,
                        kind="ExternalInput").ap()
    w = nc.dram_tensor("w", [KPAD, MPAD], mybir.dt.float32,
                       kind="ExternalInput").ap()
    b = nc.dram_tensor("b", [128, MPAD], mybir.dt.float32,
                       kind="ExternalInput").ap()
    y = nc.dram_tensor("y", [ROWS_PER_CORE_PAD, MPAD], mybir.dt.float32,
                       kind="ExternalOutput").ap()
    nblocks = ROWS_PER_CORE_PAD // 128

    with tile.TileContext(nc) as tc:
        with tc.tile_pool(name="fix", bufs=1) as fix, \
             tc.tile_pool(name="sbuf", bufs=4) as pool, \
             tc.tile_pool(name="psum", bufs=4, space="PSUM") as psum:
            w_t = fix.tile([KPAD, MPAD], mybir.dt.float32)
            b_t = fix.tile([128, MPAD], mybir.dt.float32)
            zero_t = fix.tile([128, MPAD], mybir.dt.float32)
            nc.sync.dma_start(out=w_t[:], in_=w[:])
            nc.sync.dma_start(out=b_t[:], in_=b[:])
            nc.vector.memset(zero_t[:], 0.0)
            for blk in range(nblocks):
                x_t = pool.tile([KPAD, 128], mybir.dt.float32, tag="x")
                nc.sync.dma_start(
                    out=x_t[:], in_=xt[:, blk * 128:(blk + 1) * 128])
                p_t = psum.tile([128, MPAD], mybir.dt.float32, tag="p")
                nc.tensor.matmul(out=p_t[:], lhsT=x_t[:], rhs=w_t[:],
                                 start=True, stop=True)
                o_t = pool.tile([128, MPAD], mybir.dt.float32, tag="o")
                nc.vector.tensor_tensor(
                    out=o_t[:], in0=p_t[:],
                    in1=b_t[:],
                    op=mybir.AluOpType.add)
                nc.vector.tensor_tensor(
                    out=o_t[:], in0=o_t[:], in1=zero_t[:],
                    op=mybir.AluOpType.max)
                nc.sync.dma_start(
                    out=y[blk * 128:(blk + 1) * 128, :], in_=o_t[:])
    nc.compile()
    return nc


def _device_gemm_relu(x_full, W, bias):
    """relu(x_full @ W + bias) on the 8 cores, row-sharded. x_full [N, K]."""
    from concourse import bass_utils

    if "gemm" not in _compiled:
        try:
            _compiled["gemm"] = _build_gemm_relu()
        except Exception:
            _compiled["gemm"] = None
            raise
    nc = _compiled["gemm"]
    if nc is None:
        raise RuntimeError("device GEMM unavailable (earlier build failed)")

    n, k = x_full.shape
    m = W.shape[1]
    total_pad = ROWS_PER_CORE_PAD * N_CORES
    xp = np.zeros((total_pad, KPAD), dtype=np.float32)
    xp[:n, :k] = x_full
    wp = np.zeros((KPAD, MPAD), dtype=np.float32)
    wp[:k, :m] = W
    bp = np.zeros((128, MPAD), dtype=np.float32)
    bp[:, :m] = bias

    in_maps = []
    for c in range(N_CORES):
        sl = xp[c * ROWS_PER_CORE_PAD:(c + 1) * ROWS_PER_CORE_PAD]
        in_maps.append({
            "xt": np.ascontiguousarray(sl.T),
            "w": wp,
            "b": bp,
        })
    res = bass_utils.run_bass_kernel_spmd(nc, in_maps,
                                          core_ids=list(range(N_CORES)))
    out = np.concatenate([res.results[c]["y"] for c in range(N_CORES)],
                         axis=0)
    return out[:n, :m]


def _prep_graph(edge_index):
    """Sort edges by dst; compute dinv and folded per-node constants."""
    src = edge_index[0].astype(np.int64)
    dst = edge_index[1].astype(np.int64)
    deg = np.bincount(dst, minlength=N).astype(np.float32) + 1.0  # + self loop
    dinv = 1.0 / np.sqrt(deg)
    order = np.argsort(dst, kind="stable")
    src_s = src[order]
    dst_s = dst[order]
    # segment boundaries for reduceat
    counts = np.bincount(dst_s, minlength=N)
    starts = np.zeros(N, dtype=np.int64)
    np.cumsum(counts[:-1], out=starts[1:])
    has_edges = counts > 0
    # zero-count tail nodes would index == E; they are masked by has_edges,
    # so clipping is safe and keeps reduceat in bounds.
    starts = np.minimum(starts, max(len(src_s) - 1, 0))
    return src_s, starts, has_edges, dinv


def _propagate(h, src_s, starts, has_edges, dinv):
    """APPNP propagation, K_HOPS steps, norm folded via s = dinv * x.

    x_{k+1} = (1-a) * [dinv**2 * (A's_k + s_k)] ... using s-state:
      s_{k+1} = c1 * (A' s_k + s_k) + t,  c1 = (1-a)*dinv^2, t = a*dinv*h
    where (A' s)_d = sum over non-loop edges e (dst=d) of s[src_e].
    Returns x_K = s_K / dinv.
    """
    c1 = ((1.0 - ALPHA) * dinv * dinv)[:, None].astype(np.float32)
    t = (ALPHA * dinv)[:, None].astype(np.float32) * h
    s = dinv[:, None].astype(np.float32) * h
    for _ in range(K_HOPS):
        gathered = s[src_s]                       # [E, F]
        agg = np.zeros_like(s)
        sums = np.add.reduceat(gathered, starts, axis=0)
        agg[has_edges] = sums[has_edges]
        s = c1 * (agg + s) + t
    return s / dinv[:, None]


def _log_softmax(x):
    m = x.max(axis=1, keepdims=True)
    e = np.exp(x - m)
    return (x - m) - np.log(e.sum(axis=1, keepdims=True))


def kernel(x, edge_index, W1, b1, W2, b2):
    x = np.asarray(x, dtype=np.float32)
    edge_index = np.asarray(edge_index)
    W1 = np.asarray(W1, dtype=np.float32)
    b1 = np.asarray(b1, dtype=np.float32)
    W2 = np.asarray(W2, dtype=np.float32)
    b2 = np.asarray(b2, dtype=np.float32)

    src_s, starts, has_edges, dinv = _prep_graph(edge_index)

    h1 = None
    try:
        h1 = _device_gemm_relu(x, W1, b1)
    except Exception as exc:  # device path unavailable -> numpy fallback
        print(f"kernel: device GEMM1 failed ({exc}); numpy fallback",
              file=sys.stderr)
    if h1 is None:
        h1 = np.maximum(x @ W1 + b1, 0.0)

    h1 = _propagate(h1, src_s, starts, has_edges, dinv)

    h2 = None
    try:
        h2 = _device_gemm_relu(h1, W2, b2)
    except Exception as exc:
        print(f"kernel: device GEMM2 failed ({exc}); numpy fallback",
              file=sys.stderr)
    if h2 is None:
        h2 = np.maximum(h1 @ W2 + b2, 0.0)

    h2 = _propagate(h2, src_s, starts, has_edges, dinv)
    return _log_softmax(h2).astype(np.float32)



# revision 10
# speedup vs baseline: 34.0005x; 34.0005x over previous
"""APPNP (2-layer MLP + 2x K=10 PPR propagation) fully on 8 TRN2 cores.

Design (v2 — full device, replaces the host-propagation baseline):
- Nodes padded to 100352 = 8*12544; core c owns dst rows [c*12544,(c+1)*12544).
- Scaled state m = dinv*x lives in SBUF (fp32) per core; the full node
  table (all-gathered every hop, fp32 [100352, 64], 256B rows) ping-pongs
  between two Shared DRAM buffers.
- Per hop per core: messages m[src] are fetched with GPSIMD swdge
  dma_gather (1024 int16 idxs/call, source chunked 4x25088 to fit int16)
  and accumulated into 4 per-chunk HBM accumulators with dma_scatter_add
  (idx = dst_local, pads land in trash rows >= 12544). Accumulators are
  pre-initialized with t/(4*c1) so the teleport term folds in for free:
  m' = c1*(sum_q acc_q + m). AllGather of the updated shard feeds the
  next hop.
- GEMMs, relu, and log_softmax all run on device; the kernel launches a
  single compiled program once (no per-hop host round trips).
- Numpy fallback keeps correctness if the device path fails.
"""
import os
import sys

import numpy as np

sys.path.insert(0, '/opt/trn_rl_repo')

N = 100000
E = 1600000
F_IN = 128
F_HID = 64
F_OUT = 40
K_HOPS = 10
ALPHA = 0.1

NC = 8
SHARD = 12544            # 98 * 128
NPAD = NC * SHARD        # 100352
NB = SHARD // 128        # 98 row-blocks per core
CHUNK = 25088            # NPAD / 4, < int16 max
NQ = 4
ACC_ROWS = SHARD + 256   # trash rows for padded edges
SLICE_I = 1024           # idxs per swdge call (hard ucode limit)
GB = 7                   # row-blocks per update group (98 = 14*7)

_cache = {}


# ----------------------------------------------------------------- host prep
def _preprocess(edge_index):
    src = np.asarray(edge_index[0], dtype=np.int64)
    dst = np.asarray(edge_index[1], dtype=np.int64)
    deg = np.bincount(dst, minlength=NPAD).astype(np.float32) + 1.0
    real = np.zeros(NPAD, dtype=bool)
    real[:N] = True
    dinv = np.where(real, 1.0 / np.sqrt(deg), 0.0).astype(np.float32)
    rdinv = np.where(real, np.sqrt(deg), 0.0).astype(np.float32)

    owner = dst // SHARD
    q = src // CHUNK
    key = owner * NQ + q
    # owner-major, chunk, then dst: within each (core, chunk) stream edges
    # are dst-sorted so round-robin dealing gives every 1024-idx scatter
    # call distinct dst rows (dma_scatter_add loses concurrent same-row
    # adds within a call).
    order = np.lexsort((dst, q, owner))
    ssrc = src[order]
    sdst = dst[order]
    counts = np.bincount(key, minlength=NC * NQ)
    bounds = np.zeros(NC * NQ + 1, dtype=np.int64)
    np.cumsum(counts, out=bounds[1:])
    w_max = int(np.ceil(counts.max() / SLICE_I) * SLICE_I)  # idxs per stream
    nwcols = w_max // 16
    ncall = w_max // SLICE_I

    def wrap(a):
        return np.tile(a.reshape(-1, 16).T, (8, 1)).astype(np.int16)

    def deal(a):
        # position p (dst-sorted) -> call p % ncall, slot p // ncall
        return np.ascontiguousarray(a.reshape(SLICE_I, ncall).T).reshape(-1)

    per_core = []
    trash = (12544 + (np.arange(w_max) % 256)).astype(np.int16)
    for c in range(NC):
        gws, sws = [], []
        for qq in range(NQ):
            k = c * NQ + qq
            gl = (ssrc[bounds[k]:bounds[k + 1]] % CHUNK).astype(np.int16)
            sl = (sdst[bounds[k]:bounds[k + 1]] % SHARD).astype(np.int16)
            n = len(gl)
            # max same-dst run must fit in ncall distinct calls
            if n:
                runs = np.diff(np.flatnonzero(np.r_[True, sl[1:] != sl[:-1],
                                                    True]))
                assert runs.max() <= ncall, (runs.max(), ncall)
            gpad = np.zeros(w_max, dtype=np.int16)
            gpad[:n] = gl
            spad = trash.copy()
            spad[:n] = sl
            gws.append(wrap(deal(gpad)))
            sws.append(wrap(deal(spad)))
        lo = c * SHARD
        dv = dinv[lo:lo + SHARD].reshape(NB, 128).T.copy()
        rv = rdinv[lo:lo + SHARD].reshape(NB, 128).T.copy()
        c1 = (0.9 * dv * dv).astype(np.float32)
        tpc = (0.25 * (ALPHA / 0.9) * rv).astype(np.float32)
        per_core.append({
            "gidx": np.concatenate(gws, axis=1),   # [128, NQ*nwcols] i16
            "sidx": np.concatenate(sws, axis=1),
            "dinv": np.ascontiguousarray(dv),
            "rdinv": np.ascontiguousarray(rv),
            "c1": np.ascontiguousarray(c1),
            "tpc": np.ascontiguousarray(tpc),
        })
    return per_core, w_max, nwcols


# -------------------------------------------------------------- bass program
def _build(w_max):
    from concourse import bass, bacc, tile, mybir

    f32 = mybir.dt.float32
    i16 = mybir.dt.int16
    i32 = mybir.dt.int32
    Alu = mybir.AluOpType
    Act = mybir.ActivationFunctionType

    nwcols = w_max // 16
    nslice = w_max // SLICE_I

    nc = bacc.Bacc("TRN2", target_bir_lowering=False, debug=False,
                   enable_asserts=False, num_devices=NC)

    xT = nc.dram_tensor("xT", [F_IN, SHARD], f32, kind="ExternalInput").ap()
    w1 = nc.dram_tensor("w1", [F_IN, F_HID], f32, kind="ExternalInput").ap()
    b1 = nc.dram_tensor("b1", [128, F_HID], f32, kind="ExternalInput").ap()
    w2 = nc.dram_tensor("w2", [F_HID, F_HID], f32, kind="ExternalInput").ap()
    b2 = nc.dram_tensor("b2", [128, F_HID], f32, kind="ExternalInput").ap()
    gidx = nc.dram_tensor("gidx", [128, NQ * nwcols], i16,
                          kind="ExternalInput").ap()
    sidx = nc.dram_tensor("sidx", [128, NQ * nwcols], i16,
                          kind="ExternalInput").ap()
    dinv = nc.dram_tensor("dinv", [128, NB], f32, kind="ExternalInput").ap()
    rdinv = nc.dram_tensor("rdinv", [128, NB], f32, kind="ExternalInput").ap()
    c1 = nc.dram_tensor("c1", [128, NB], f32, kind="ExternalInput").ap()
    tpc = nc.dram_tensor("tpc", [128, NB], f32, kind="ExternalInput").ap()
    y = nc.dram_tensor("y", [SHARD, F_OUT], f32, kind="ExternalOutput").ap()

    tabA = nc.dram_tensor("tabA", [NPAD, F_HID], f32, kind="Internal",
                          addr_space="Shared").ap()
    tabB = nc.dram_tensor("tabB", [NPAD, F_HID], f32, kind="Internal",
                          addr_space="Shared").ap()
    bounce = nc.dram_tensor("bounce", [SHARD, F_HID], f32,
                            kind="Internal").ap()
    accs = [nc.dram_tensor(f"acc{a}", [ACC_ROWS, F_HID], f32,
                           kind="Internal").ap() for a in range(NQ)]
    tabs = [tabA, tabB]

    with tile.TileContext(nc) as tc:
        with tc.tile_pool(name="fix", bufs=1) as fix, \
             tc.tile_pool(name="state", bufs=1) as stp, \
             tc.tile_pool(name="sbuf", bufs=8) as pool, \
             tc.tile_pool(name="upd", bufs=2) as updp, \
             tc.tile_pool(name="psum", bufs=4, space="PSUM") as psum:

            # ---- fixed tiles
            gidx_t = fix.tile([128, NQ * nwcols], i16)
            nc.sync.dma_start(out=gidx_t[:], in_=gidx[:])
            sidx_t = fix.tile([128, NQ * nwcols], i16)
            nc.sync.dma_start(out=sidx_t[:], in_=sidx[:])
            dinv_t = fix.tile([128, NB], f32)
            nc.sync.dma_start(out=dinv_t[:], in_=dinv[:])
            rdinv_t = fix.tile([128, NB], f32)
            nc.sync.dma_start(out=rdinv_t[:], in_=rdinv[:])
            c1_t = fix.tile([128, NB], f32)
            nc.sync.dma_start(out=c1_t[:], in_=c1[:])
            tpc_t = fix.tile([128, NB], f32)
            nc.sync.dma_start(out=tpc_t[:], in_=tpc[:])
            w1_t = fix.tile([F_IN, F_HID], f32)
            nc.sync.dma_start(out=w1_t[:], in_=w1[:])
            b1_t = fix.tile([128, F_HID], f32)
            nc.sync.dma_start(out=b1_t[:], in_=b1[:])
            w2_t = fix.tile([F_HID, F_HID], f32)
            nc.sync.dma_start(out=w2_t[:], in_=w2[:])
            b2_t = fix.tile([128, F_HID], f32)
            nc.sync.dma_start(out=b2_t[:], in_=b2[:])

            # identity for PE transpose
            iota_r = fix.tile([128, 128], i32)
            nc.gpsimd.iota(iota_r[:], pattern=[[1, 128]], base=0,
                           channel_multiplier=0)
            iota_rf = fix.tile([128, 128], f32)
            nc.vector.tensor_scalar_add(iota_rf[:], iota_r[:], 0.0)
            iota_c = fix.tile([128, 1], i32)
            nc.gpsimd.iota(iota_c[:], pattern=[[0, 1]], base=0,
                           channel_multiplier=1)
            iota_cf = fix.tile([128, 1], f32)
            nc.vector.tensor_scalar_add(iota_cf[:], iota_c[:], 0.0)
            ident_t = fix.tile([128, 128], f32)
            nc.vector.tensor_scalar(out=ident_t[:], in0=iota_rf[:],
                                    scalar1=iota_cf[:], scalar2=None,
                                    op0=Alu.is_equal)

            # persistent state
            m_t = stp.tile([128, NB, F_HID], f32)     # m = dinv * x
            tq_t = stp.tile([128, NB, F_HID], f32)    # t_pre / 4

            def allgather(tab_dst):
                nc.sync.dma_start(
                    out=bounce.rearrange("(b p) f -> p b f", p=128),
                    in_=m_t[:])
                nc.gpsimd.collective_compute(
                    "AllGather", Alu.bypass,
                    replica_groups=[list(range(NC))],
                    ins=[bounce.opt()], outs=[tab_dst.opt()])

            # ---- GEMM1: h1 = relu(x @ W1 + b1); m0 = dinv*h1; tq = tpc*h1
            for b in range(NB):
                lx = pool.tile([F_IN, 128], f32, tag="lx")
                nc.sync.dma_start(out=lx[:], in_=xT[:, b * 128:(b + 1) * 128])
                pt = psum.tile([128, F_HID], f32, tag="pg")
                nc.tensor.matmul(out=pt[:], lhsT=lx[:], rhs=w1_t[:],
                                 start=True, stop=True)
                h = pool.tile([128, F_HID], f32, tag="h")
                nc.vector.tensor_tensor(out=h[:], in0=pt[:], in1=b1_t[:],
                                        op=Alu.add)
                nc.vector.tensor_scalar_max(h[:], h[:], 0.0)
                nc.vector.tensor_scalar_mul(m_t[:, b, :], h[:],
                                            dinv_t[:, b:b + 1])
                nc.vector.tensor_scalar_mul(tq_t[:, b, :], h[:],
                                            tpc_t[:, b:b + 1])
            allgather(tabA)

            # ---- one propagation hop
            def hop(tsrc, tdst, do_ag=True):
                for a in range(NQ):
                    nc.sync.dma_start(
                        out=accs[a][0:SHARD, :].rearrange(
                            "(b p) f -> p b f", p=128),
                        in_=tq_t[:])
                for s in range(nslice):
                    for a in range(NQ):
                        cols = slice(a * nwcols + s * (SLICE_I // 16),
                                     a * nwcols + (s + 1) * (SLICE_I // 16))
                        g = pool.tile([128, SLICE_I // 128, F_HID], f32,
                                      tag="msg")
                        nc.gpsimd.dma_gather(
                            g[:], tsrc[a * CHUNK:(a + 1) * CHUNK, :],
                            gidx_t[:, cols], SLICE_I, SLICE_I, F_HID)
                        nc.gpsimd.dma_scatter_add(
                            accs[a], g[:], sidx_t[:, cols],
                            SLICE_I, SLICE_I, F_HID)
                for gi in range(NB // GB):
                    rows = slice(gi * GB * 128, (gi + 1) * GB * 128)
                    ats = []
                    for a in range(NQ):
                        at = updp.tile([128, GB, F_HID], f32, tag=f"a{a}")
                        nc.sync.dma_start(
                            out=at[:],
                            in_=accs[a][rows, :].rearrange(
                                "(b p) f -> p b f", p=128))
                        ats.append(at)
                    s1 = updp.tile([128, GB, F_HID], f32, tag="s1")
                    nc.vector.tensor_tensor(out=s1[:], in0=ats[0][:],
                                            in1=ats[1][:], op=Alu.add)
                    s2 = updp.tile([128, GB, F_HID], f32, tag="s2")
                    nc.vector.tensor_tensor(out=s2[:], in0=ats[2][:],
                                            in1=ats[3][:], op=Alu.add)
                    nc.vector.tensor_tensor(out=s1[:], in0=s1[:], in1=s2[:],
                                            op=Alu.add)
                    nc.vector.tensor_tensor(
                        out=s1[:], in0=s1[:],
                        in1=m_t[:, gi * GB:(gi + 1) * GB, :], op=Alu.add)
                    for j in range(GB):
                        b = gi * GB + j
                        nc.vector.tensor_scalar_mul(
                            m_t[:, b, :], s1[:, j, :], c1_t[:, b:b + 1])
                if do_ag:
                    allgather(tdst)

            for h_i in range(K_HOPS):
                hop(tabs[h_i % 2], tabs[(h_i + 1) % 2],
                    do_ag=(h_i < K_HOPS - 1))

            # ---- GEMM2: x10 = m*rdinv; h2 = relu(x10 @ W2 + b2)
            for b in range(NB):
                xb = pool.tile([128, F_HID], f32, tag="xb")
                nc.vector.tensor_scalar_mul(xb[:], m_t[:, b, :],
                                            rdinv_t[:, b:b + 1])
                ptr = psum.tile([F_HID, 128], f32, tag="ptr")
                nc.tensor.transpose(ptr[:], xb[:], ident_t[:])
                lT = pool.tile([F_HID, 128], f32, tag="lT")
                nc.scalar.copy(out=lT[:], in_=ptr[:])
                p2 = psum.tile([128, F_HID], f32, tag="pg")
                nc.tensor.matmul(out=p2[:], lhsT=lT[:], rhs=w2_t[:],
                                 start=True, stop=True)
                h = pool.tile([128, F_HID], f32, tag="h")
                nc.vector.tensor_tensor(out=h[:], in0=p2[:], in1=b2_t[:],
                                        op=Alu.add)
                nc.vector.tensor_scalar_max(h[:], h[:], 0.0)
                nc.vector.tensor_scalar_mul(m_t[:, b, :], h[:],
                                            dinv_t[:, b:b + 1])
                nc.vector.tensor_scalar_mul(tq_t[:, b, :], h[:],
                                            tpc_t[:, b:b + 1])
            allgather(tabA)

            for h_i in range(K_HOPS):
                hop(tabs[h_i % 2], tabs[(h_i + 1) % 2],
                    do_ag=(h_i < K_HOPS - 1))

            # ---- log_softmax over first F_OUT cols
            for b in range(NB):
                xf = pool.tile([128, F_OUT], f32, tag="xf")
                nc.vector.tensor_scalar_mul(xf[:], m_t[:, b, 0:F_OUT],
                                            rdinv_t[:, b:b + 1])
                mx = pool.tile([128, 1], f32, tag="mx")
                nc.vector.tensor_reduce(out=mx[:], in_=xf[:],
                                        axis=mybir.AxisListType.X, op=Alu.max)
                mxn = pool.tile([128, 1], f32, tag="mxn")
                nc.vector.tensor_scalar_mul(mxn[:], mx[:], -1.0)
                ex = pool.tile([128, F_OUT], f32, tag="ex")
                nc.scalar.activation(out=ex[:], in_=xf[:], func=Act.Exp,
                                     bias=mxn[:])
                sm = pool.tile([128, 1], f32, tag="sm")
                nc.vector.tensor_reduce(out=sm[:], in_=ex[:],
                                        axis=mybir.AxisListType.X, op=Alu.add)
                ls = pool.tile([128, 1], f32, tag="ls")
                nc.scalar.activation(out=ls[:], in_=sm[:], func=Act.Ln)
                fin = pool.tile([128, F_OUT], f32, tag="fin")
                nc.vector.tensor_scalar(out=fin[:], in0=xf[:],
                                        scalar1=mx[:], scalar2=ls[:],
                                        op0=Alu.subtract, op1=Alu.subtract)
                nc.sync.dma_start(out=y[b * 128:(b + 1) * 128, :], in_=fin[:])
    nc.compile()
    return nc


# ------------------------------------------------------------------- driver
def _run_device(inputs, trace=False):
    from concourse import bass_utils

    x = np.asarray(inputs["x"], dtype=np.float32)
    W1 = np.asarray(inputs["W1"], dtype=np.float32)
    b1 = np.asarray(inputs["b1"], dtype=np.float32)
    W2 = np.asarray(inputs["W2"], dtype=np.float32)
    b2 = np.asarray(inputs["b2"], dtype=np.float32)
    edge_index = np.asarray(inputs["edge_index"])

    per_core, w_max, _ = _preprocess(edge_index)
    key = ("prog", w_max)
    if key not in _cache:
        _cache[key] = _build(w_max)
    nc = _cache[key]

    xp = np.zeros((NPAD, F_IN), dtype=np.float32)
    xp[:N] = x
    w1p = W1
    b1p = np.tile(b1[None, :], (128, 1)).astype(np.float32)
    w2p = np.zeros((F_HID, F_HID), dtype=np.float32)
    w2p[:, :F_OUT] = W2
    b2p = np.zeros((128, F_HID), dtype=np.float32)
    b2p[:, :F_OUT] = b2

    in_maps = []
    for c in range(NC):
        pc = per_core[c]
        in_maps.append({
            "xT": np.ascontiguousarray(xp[c * SHARD:(c + 1) * SHARD].T),
            "w1": w1p, "b1": b1p, "w2": w2p, "b2": b2p,
            "gidx": pc["gidx"], "sidx": pc["sidx"],
            "dinv": pc["dinv"], "rdinv": pc["rdinv"],
            "c1": pc["c1"], "tpc": pc["tpc"],
        })
    res = bass_utils.run_bass_kernel_spmd(
        nc, in_maps, core_ids=list(range(NC)), trace=trace)
    out = np.concatenate([res.results[c]["y"] for c in range(NC)], axis=0)
    return out[:N], res


# ------------------------------------------------------------ numpy fallback
def _numpy_ref(x, edge_index, W1, b1, W2, b2):
    src = edge_index[0].astype(np.int64)
    dst = edge_index[1].astype(np.int64)
    deg = np.bincount(dst, minlength=N).astype(np.float32) + 1.0
    dinv = 1.0 / np.sqrt(deg)

    def prop(h):
        m = dinv[:, None] * h
        c1 = 0.9 * dinv * dinv
        t = ALPHA * dinv[:, None] * h
        for _ in range(K_HOPS):
            agg = np.zeros_like(m)
            np.add.at(agg, dst, m[src])
            m = c1[:, None] * (agg + m) + t
        return m / dinv[:, None]

    h = np.maximum(x @ W1 + b1, 0.0)
    h = prop(h)
    h = np.maximum(h @ W2 + b2, 0.0)
    h = prop(h)
    mx = h.max(axis=1, keepdims=True)
    e = np.exp(h - mx)
    return (h - mx) - np.log(e.sum(axis=1, keepdims=True))


def kernel(x, edge_index, W1, b1, W2, b2):
    inputs = {"x": x, "edge_index": edge_index, "W1": W1, "b1": b1,
              "W2": W2, "b2": b2}
    try:
        out, _ = _run_device(inputs, trace=False)
        return out.astype(np.float32)
    except Exception as exc:  # device path failed -> numpy fallback
        print(f"kernel: device path failed ({exc!r}); numpy fallback",
              file=sys.stderr)
        return _numpy_ref(np.asarray(x, np.float32), np.asarray(edge_index),
                          np.asarray(W1, np.float32),
                          np.asarray(b1, np.float32),
                          np.asarray(W2, np.float32),
                          np.asarray(b2, np.float32)).astype(np.float32)
